# revision 1
# baseline (speedup 1.0000x reference)
"""ColumnParallelLinearWithDelta: GPTQ-int4 LoRA-delta matmul on 8 trn2 cores.

out[d] = x @ dequant(qweight[d], qzeros[d], scales[d]) + x @ base_weight.T

Sharding: column-parallel — out_features (4096) split into 8 slices of 512,
one per NeuronCore; x replicated. Each core computes its [8, 256, 512] slice
of the delta stack plus the shared base output; the host adds base during
the unshard (exact f32 broadcast-add, O(output) work).

Math (per core, out-col slice ns):
  W[k, n]  = s[g(k), n] * (w4[k, n] - (z4[g(k), n] + 1)),  g(k) = k // 128
  delta    = x @ W = x @ (s .* w4)  -  xs @ (s .* (z4 + 1))
  with xs[t, g] = sum_{k in g} x[t, k]   (host-precomputed group sums)
  out[d]   = delta_d + base,  base = x @ base_weight[ns, :].T

Layout trick (all host-side shuffles, nothing extra on device): each packed
int32 row r holds nibbles j = 0..7 of input rows k = 8r+j; viewing it as
int16, the even halfwords hold j = 0..3 and the odd halfwords j = 4..7 of
the same output column n. The host splits each row's even/odd halfwords
across two partition halves (p = 64*e + r64, 64-row chunks c), so ONE
tensor_scalar (>> 4sh) & 0xF over the whole [128, 4096] tile yields, per
shift sh, a DENSE [128, 512] nibble plane per chunk: partitions p < 64
carry j = sh, partitions p >= 64 carry j = 4+sh, free dim = n (step 1).
Each plane is multiplied by a matching partition-replicated scale tile
(tensor_tensor, fp16 2x mode) and consumed directly as a matmul moving
operand against a host-reordered stationary x tile — contraction rows
k(p) = 8*(64c + p%64) + 4*(p//64) + sh.

Per adapter: 4 TS + 4 TT over [128, 4096] on VectorE, 64 matmuls (8 chunks
x 4 shifts x 2 t-halves) accumulating f32 in PSUM, one K=32 correction
matmul (-xs^T x (s*(z4+1))), ScalarE PSUM->SBUF drain, DMA out. The base
matmul streams similarly from fp16 base weights (DMA only, no dequant).
"""

import numpy as np

# ---- problem constants (hardcoded; kernel.py must be self-contained) ----
T = 256          # tokens
IN = 4096        # in_features
OUT = 4096       # out_features
D = 8            # adapters
GROUP = 128      # quant group size
G = IN // GROUP  # 32 groups
NCORES = 8
NC_OUT = OUT // NCORES   # 512 out cols per core
NCH = 8                  # contraction chunks of 64 packed rows (x2 e-halves)

_PROGRAM_CACHE: dict = {}


def _build_program():
    import concourse.bacc as bacc
    import concourse.mybir as mybir
    import concourse.tile as tile

    nc = bacc.Bacc("TRN2", target_bir_lowering=False, debug=False)

    fp16 = mybir.dt.float16
    FD = NCH * NC_OUT        # 4096 halfwords per partition per adapter
    d_xt = nc.dram_tensor("xt", (128, NCH * 4 * T), fp16, kind="ExternalInput")
    d_negxs = nc.dram_tensor("negxs", (G, T), fp16, kind="ExternalInput")
    d_qw16 = nc.dram_tensor(
        "qw16", (D, 2, 128, FD // 2), mybir.dt.int16, kind="ExternalInput"
    )
    d_s2 = nc.dram_tensor("s2", (D, 2, 128, FD // 2), fp16,
                          kind="ExternalInput")
    d_wb = nc.dram_tensor("wb", (NCH, 128, 4 * NC_OUT), fp16,
                          kind="ExternalInput")
    d_sz = nc.dram_tensor("sz", (G, D * NC_OUT), fp16, kind="ExternalInput")
    d_out = nc.dram_tensor("out", (D, T, NC_OUT), mybir.dt.float32,
                           kind="ExternalOutput")
    d_outb = nc.dram_tensor("outb", (T, NC_OUT), mybir.dt.float32,
                            kind="ExternalOutput")

    AT = mybir.AluOpType

    with tile.TileContext(nc) as tc:
        with (
            tc.tile_pool(name="const", bufs=1) as cpool,
            tc.tile_pool(name="qw", bufs=2) as qpool,
            tc.tile_pool(name="s2", bufs=2) as spool,
            tc.tile_pool(name="vr", bufs=1) as vrpool,
            tc.tile_pool(name="v", bufs=2) as vpool,
            tc.tile_pool(name="wb", bufs=2) as wpool,
            tc.tile_pool(name="outp", bufs=4) as opool,
            tc.tile_pool(name="ps", bufs=2, space="PSUM") as ppool,
            tc.tile_pool(name="psb", bufs=1, space="PSUM") as pbpool,
        ):
            xt_sb = cpool.tile([128, NCH * 4 * T], fp16)
            negxs_sb = cpool.tile([G, T], fp16)
            sz_sb = cpool.tile([G, D * NC_OUT], fp16)
            warm_sb = cpool.tile([128, 640], fp16)

            def xt_tile(c, sh, th):
                off = (c * 4 + sh) * T + th * 128
                return xt_sb[:, off:off + 128]

            def load_adapter(d):
                """DMA an adapter's packed weights + scales as two
                DRAM-contiguous half blocks each (qw/s2 interleaved so the
                first dequant pair lands first)."""
                qw_t = qpool.tile([128, FD], mybir.dt.int16, name="qw_t")
                s2_t = spool.tile([128, FD], fp16, name="s2_t")
                hs = FD // 2
                for h in range(2):
                    nc.sync.dma_start(qw_t[:, h * hs:(h + 1) * hs],
                                      d_qw16[d, h, :, :])
                    s2_dma = nc.sync.dma_start(s2_t[:, h * hs:(h + 1) * hs],
                                               d_s2[d, h, :, :])
                    if d == 0 and h == 0:
                        d0_s2_dma[0] = s2_dma
                return qw_t, s2_t

            def adapter_main(d, ps, loaded=None, n_parts=1):
                """Dequant (4 TS + 4 TT over [128, 4096]) + 64 matmuls.

                n_parts > 1 splits the dequant ops along the free dim
                (chunk-granular) so the first planes are ready sooner —
                used for adapter 0 to shorten the startup critical path."""
                qw_t, s2_t = loaded if loaded else load_adapter(d)
                cs = FD // n_parts
                ch_per = NCH // n_parts
                for part in range(n_parts):
                    c0 = part * cs
                    for sh in range(4):
                        vr = vrpool.tile([128, FD], mybir.dt.int16,
                                         tag=f"vr{sh}", name=f"vr{sh}")
                        ts_i = nc.vector.tensor_scalar(
                            out=vr[:, c0:c0 + cs], in0=qw_t[:, c0:c0 + cs],
                            scalar1=4 * sh, scalar2=0xF,
                            op0=AT.logical_shift_right, op1=AT.bitwise_and,
                        )
                        v = vpool.tile([128, FD], fp16, tag=f"v{sh}",
                                       name=f"v{sh}")
                        tt_i = nc.vector.tensor_tensor(
                            out=v[:, c0:c0 + cs], in0=vr[:, c0:c0 + cs],
                            in1=s2_t[:, c0:c0 + cs], op=AT.mult
                        )
                        if d == 0 and part == 0 and sh == 0:
                            first_ops[0] = (ts_i, tt_i)
                        for c in range(part * ch_per, (part + 1) * ch_per):
                            rhs = v[:, c * NC_OUT:(c + 1) * NC_OUT]
                            for th in range(2):
                                nc.tensor.matmul(
                                    ps[th][:],
                                    lhsT=xt_tile(c, sh, th),
                                    rhs=rhs,
                                    start=(part == 0 and sh == 0 and c == 0),
                                    stop=False,
                                )

            def adapter_finish(d, ps):
                """Zeros correction; ScalarE drains PSUM (base added on
                host during unshard)."""
                for th in range(2):
                    nc.tensor.matmul(
                        ps[th][:],
                        lhsT=negxs_sb[:, th * 128:(th + 1) * 128],
                        rhs=sz_sb[:, d * NC_OUT:(d + 1) * NC_OUT],
                        start=False, stop=True,
                    )
                for th in range(2):
                    o_t = opool.tile([128, NC_OUT], mybir.dt.float32, name="o_t")
                    nc.scalar.copy(o_t[:], ps[th][:])
                    # ACT's HWDGE queue: keeps output writes off the Sync
                    # queue that feeds the weight-stream DMAs
                    nc.scalar.dma_start(
                        d_out[d, th * 128:(th + 1) * 128, :], o_t[:]
                    )

            # ---- order: short PE warm-up, adapter 0 (split dequant so the
            # PE starts early), base matmuls (DMA-only, fills the window
            # while DVE dequants adapter 1), then adapters 1..7.
            first_ops = [None]
            d0_s2_dma = [None]
            loaded0 = load_adapter(0)
            # xt on the ACT HWDGE queue (off the weight queue); xt halves 2-3
            # and the small consts are gated behind the first dequant op so
            # the startup-critical qw/s2/xt0-1 transfers get the bandwidth
            late_dmas = []
            for q in range(4):
                dma = nc.scalar.dma_start(
                    xt_sb[:, q * 8 * T:(q + 1) * 8 * T],
                    d_xt[:, q * 8 * T:(q + 1) * 8 * T])
                if q >= 2:
                    late_dmas.append(dma)
            late_dmas.append(nc.scalar.dma_start(negxs_sb[:], d_negxs[:]))
            late_dmas.append(nc.scalar.dma_start(sz_sb[:], d_sz[:]))

            # PE warm-up: dummy matmuls flip the HAM clock gate to 2.4GHz
            # while the first weight transfers land, so the real stream
            # starts warm
            nc.gpsimd.memset(warm_sb[:], 0.0)
            with tc.tile_pool(name="warmps", bufs=1, space="PSUM") as wpsp:
                warm_ps = wpsp.tile([128, NC_OUT], mybir.dt.float32)
                for _ in range(22):
                    nc.tensor.matmul(
                        warm_ps[:], lhsT=warm_sb[:, :128], rhs=warm_sb[:, 128:],
                        start=True, stop=True,
                    )

            ps0 = [ppool.tile([128, NC_OUT], mybir.dt.float32, tag=f"ps{t}",
                              name=f"ps{t}") for t in range(2)]
            adapter_main(0, ps0, loaded=loaded0, n_parts=2)
            for dma in late_dmas:
                tile.add_dep_helper(dma.ins, first_ops[0][0].ins,
                                    reason="gate non-urgent startup DMAs")

            ps_b = [pbpool.tile([128, NC_OUT], mybir.dt.float32, tag=f"psb{t}",
                                name=f"psb{t}") for t in range(2)]
            for c in range(NCH):
                wb_t = wpool.tile([128, 4 * NC_OUT], fp16, name="wb_t",
                                  tag="wb")
                wb_dma = nc.sync.dma_start(wb_t[:], d_wb[c, :, :])
                tile.add_dep_helper(wb_dma.ins, first_ops[0][1].ins,
                                    reason="gate base-weight stream")
                for sh in range(4):
                    rhs = wb_t[:, sh * NC_OUT:(sh + 1) * NC_OUT]
                    for th in range(2):
                        nc.tensor.matmul(
                            ps_b[th][:],
                            lhsT=xt_tile(c, sh, th),
                            rhs=rhs,
                            start=(c == 0 and sh == 0),
                            stop=(c == NCH - 1 and sh == 3),
                        )
            adapter_finish(0, ps0)
            for th in range(2):
                ob_t = opool.tile([128, NC_OUT], mybir.dt.float32, name="ob_t")
                nc.scalar.copy(ob_t[:], ps_b[th][:])
                nc.scalar.dma_start(
                    d_outb[th * 128:(th + 1) * 128, :], ob_t[:]
                )

            for d in range(1, D):
                ps = [ppool.tile([128, NC_OUT], mybir.dt.float32, tag=f"ps{t}",
                                 name=f"ps{t}") for t in range(2)]
                adapter_main(d, ps)
                adapter_finish(d, ps)

    nc.compile()
    return nc


def _prep_inputs(x, base_weight, qweight, qzeros, scales):
    """Host-side layout prep. Returns list of 8 per-core input maps."""
    x = np.asarray(x, dtype=np.float32)
    base_weight = np.asarray(base_weight, dtype=np.float32)
    qweight = np.asarray(qweight, dtype=np.int32)
    qzeros = np.asarray(qzeros, dtype=np.int32)
    scales = np.asarray(scales, dtype=np.float32)

    # stationary x tiles: xt[64e + r64, (4c+sh)*T + t] = x[t, 8*(64c+r64)
    # + 4e + sh]  — matches the dense nibble-plane partition layout
    xr = np.ascontiguousarray(x.T).reshape(NCH, 64, 2, 4, T)  # [c,r64,e,sh,t]
    xt = np.ascontiguousarray(xr.transpose(2, 1, 0, 3, 4))    # [e,r64,c,sh,t]
    xt = xt.reshape(128, NCH * 4 * T).astype(np.float16)

    # group sums of x (for the zeros-correction contraction), negated
    xs = x.reshape(T, G, GROUP).sum(axis=2)                   # [t, g]
    negxs = np.ascontiguousarray((-xs.T)).astype(np.float16)  # [g, t]

    # unpack qzeros (packed along out cols): z4[d, g, 8m+jj]
    jj = 4 * np.arange(8, dtype=np.int32)
    z4 = ((qzeros[:, :, :, None] >> jj[None, None, None, :]) & 0xF)
    z4 = z4.reshape(D, G, OUT)                                # [d, g, n]
    sz_full = scales * (z4 + 1).astype(np.float32)            # [d, g, n]

    in_maps = []
    for c in range(NCORES):
        ns = slice(c * NC_OUT, (c + 1) * NC_OUT)

        # packed weights: partition p = 64e + r64 holds the e-half words of
        # packed row 64c + r64; free = chunk-major, n dense within chunk
        qw_c = np.ascontiguousarray(qweight[:, :, ns])        # [D, 512, 512]
        qw16 = qw_c.view(np.int16).reshape(D, NCH, 64, NC_OUT, 2)
        qw16 = np.ascontiguousarray(qw16.transpose(0, 4, 2, 1, 3))
        # [d, e, r64, c, n] -> [D, 2 halves, 128, FD/2]
        qw16 = qw16.reshape(D, 128, NCH * NC_OUT)
        qw16 = qw16.reshape(D, 128, 2, NCH * NC_OUT // 2).transpose(0, 2, 1, 3)

        # scale tile: s2[d, p, c*512+n] = s[d, 4c + (p%64)//16, n]
        s_c = scales[:, :, ns]                                # [D, G, 512]
        s2 = s_c.reshape(D, NCH, 4, NC_OUT)                   # [d, c, g4, n]
        s2 = np.repeat(s2, 16, axis=2)                        # [d, c, 64, n]
        s2 = np.broadcast_to(s2[:, None], (D, 2, NCH, 64, NC_OUT))
        s2 = np.ascontiguousarray(s2.transpose(0, 1, 3, 2, 4))  # [d,e,r64,c,n]
        s2 = s2.reshape(D, 128, NCH * NC_OUT).astype(np.float16)
        s2 = s2.reshape(D, 128, 2, NCH * NC_OUT // 2).transpose(0, 2, 1, 3)

        # base weights in the same sub-chunk order: wb[c][p, sh*512+n] =
        # base_weight[ns, :].T[k(p, c, sh), n]
        bw_c = base_weight[ns, :]                             # [512, 4096]
        wb = np.ascontiguousarray(bw_c.T).reshape(NCH, 64, 2, 4, NC_OUT)
        wb = np.ascontiguousarray(wb.transpose(0, 2, 1, 3, 4))  # [c,e,r64,sh,n]
        wb = wb.reshape(NCH, 128, 4 * NC_OUT).astype(np.float16)

        sz_c = sz_full[:, :, ns]                              # [D, G, 512]
        sz = np.ascontiguousarray(sz_c.transpose(1, 0, 2)).reshape(G, D * NC_OUT)
        sz = sz.astype(np.float16)

        in_maps.append({
            "xt": xt, "negxs": negxs,
            "qw16": np.ascontiguousarray(qw16),
            "s2": np.ascontiguousarray(s2),
            "wb": np.ascontiguousarray(wb),
            "sz": sz,
        })
    return in_maps


def _run(in_maps, trace=False):
    from concourse import bass_utils
    if "nc" not in _PROGRAM_CACHE:
        _PROGRAM_CACHE["nc"] = _build_program()
    nc = _PROGRAM_CACHE["nc"]
    res = bass_utils.run_bass_kernel_spmd(
        nc, in_maps, core_ids=list(range(NCORES)), trace=trace
    )
    return res


def kernel(x, base_weight, qweight, qzeros, scales, g_idx, _trace=False,
           _return_results=False):
    in_maps = _prep_inputs(x, base_weight, qweight, qzeros, scales)
    res = _run(in_maps, trace=_trace)
    out = np.concatenate(
        [res.results[c]["out"] + res.results[c]["outb"][None, :, :]
         for c in range(NCORES)], axis=2)
    if _return_results:
        return out, res
    return out

